# revision 7
# baseline (speedup 1.0000x reference)
"""LoRA cross-attention Trainium2 kernel.

Reference computation (per batch b):
    qh = (q @ Wq.T + (q @ Aq.T) @ Bq.T / 8) . reshape heads          [N1, H, D]
    kh = x @ Wk.T ;  vh = x @ Wv.T + (x @ Av.T) @ Bv.T / 8
    attn = softmax(qh @ kh.T * D**-0.5)                              [H, N1, N2]
    out  = (attn @ vh) @ Wp.T + bp                                   [N1, C]
returns (out, attn).

Sharding: data parallel — one batch element per NeuronCore (8 batches, 8 cores).
Weights replicated. Device kernel computes everything for its batch; the attn
matrix is produced transposed per head ([n2, n1]) and the projection output
transposed ([C, n1]); host transposes views back.

Device-side layout choices:
  - all activations kept "feature on partitions": qhT/khT = [C=768, N] as
    6 partition-tiles of 128 (= 2 heads of 64 per tile).
  - scores^T computed per head with K=64 matmuls (row-halves of the PE array),
    exp on ScalarE (PSUM->SBUF), row sums obtained for free by augmenting V
    with a ones column in the attn@V matmul (M=65), reciprocal on DVE,
    broadcast of recip row across partitions with one fp32 PE matmul against
    a constant selector matrix, normalization on DVE.
  - matmuls run in float32r (1 cycle/row when free dim >= 256; plain fp32 is
    4 cycles/row). The recip broadcast matmul stays fp32 for exactness.
"""

import numpy as np

import concourse.bass as bass
import concourse.tile as tile
from concourse import bacc, mybir
from concourse.bass import ds, ts
from concourse.bass_utils import run_bass_kernel_spmd

F32 = mybir.dt.float32
F32R = mybir.dt.float32r
AF = mybir.ActivationFunctionType
MULT = mybir.AluOpType.mult

DIM = 768
HEADS = 12
HD = 64
RANK = 8
N = 1024  # N1 == N2
NT = DIM // 128  # 6 partition-tiles of the feature dim
NCORES = 8
SCALE = HD ** -0.5
LORA_SCALE = 1.0 / RANK


def emit_kernel(tc, ins, outs):
    from contextlib import ExitStack

    with ExitStack() as ctx:
        _emit_kernel(ctx, tc, ins, outs)


def _emit_kernel(ctx, tc, ins, outs):
    nc = tc.nc
    qT_d = ins["qT"]
    xT_d = ins["xT"]
    attnT_d = outs["attnT"]
    outT_d = outs["outT"]

    persist = ctx.enter_context(tc.tile_pool(name="persist", bufs=1))
    qhT = persist.tile([128, NT, N], F32R)   # scaled q heads^T  [c, n1]
    khT = persist.tile([128, NT, N], F32R)   # k heads^T         [c, n2]
    vha = persist.tile([128, 8, HEADS, HD + 1], F32R)  # v natural [n2, h, d] + ones col
    wp_sb = persist.tile([128, NT, DIM], F32R)
    bp_sb = persist.tile([128, NT], F32)
    one65 = persist.tile([HD + 1, 128], F32)  # selector: row 64 ones, rest zero

    nc.sync.dma_start(wp_sb[:], ins["wpT"].rearrange("(t p) o -> p t o", p=128))
    nc.sync.dma_start(bp_sb[:], ins["bpP"][:])
    # vha ones column: memset can't write f32r, so go through ScalarE
    # (out = Copy(in*1 + 1.0) with a zeroed f32 scratch input).
    zeros96 = persist.tile([128, 8 * HEADS], F32)
    nc.vector.memset(zeros96[:, :], 0.0)
    nc.scalar.activation(
        vha[:, :, :, HD],
        zeros96.rearrange("p (a b) -> p a b", b=HEADS),
        AF.Copy,
        bias=1.0,
        scale=1.0,
    )
    nc.vector.memset(one65[:, :], 0.0)
    nc.vector.memset(one65[HD : HD + 1, :], 1.0)

    # ---------------- Phase A: projections ----------------
    with tc.tile_pool(name="ph1", bufs=1) as ph1, \
         tc.tile_pool(name="psA", bufs=4, space="PSUM") as psA:
        qT_sb = ph1.tile([128, NT, N], F32R)
        xT_sb = ph1.tile([128, NT, N], F32R)
        wq_sb = ph1.tile([128, NT, DIM], F32R)
        wk_sb = ph1.tile([128, NT, DIM], F32R)
        wv_sb = ph1.tile([128, NT, DIM], F32R)
        aq_sb = ph1.tile([128, NT, RANK], F32R)
        av_sb = ph1.tile([128, NT, RANK], F32R)
        bq_sb = ph1.tile([RANK, DIM], F32R)
        bv_sb = ph1.tile([RANK, DIM], F32R)
        uq_sb = ph1.tile([RANK, N], F32R)
        uv_sb = ph1.tile([RANK, N], F32R)

        nc.sync.dma_start(qT_sb[:], qT_d.rearrange("(t p) n -> p t n", p=128))
        nc.sync.dma_start(xT_sb[:], xT_d.rearrange("(t p) n -> p t n", p=128))
        nc.sync.dma_start(wq_sb[:], ins["wqT"].rearrange("(t p) o -> p t o", p=128))
        nc.sync.dma_start(wk_sb[:], ins["wkT"].rearrange("(t p) o -> p t o", p=128))
        nc.sync.dma_start(wv_sb[:], ins["wvT"].rearrange("(t p) o -> p t o", p=128))
        nc.sync.dma_start(aq_sb[:], ins["aqT"].rearrange("(t p) r -> p t r", p=128))
        nc.sync.dma_start(av_sb[:], ins["avT"].rearrange("(t p) r -> p t r", p=128))
        nc.sync.dma_start(bq_sb[:], ins["bqT"][:])
        nc.sync.dma_start(bv_sb[:], ins["bvT"][:])

        # LoRA down-projections: u = A @ t^T  -> [rank, N]
        for u_sb, a_sb, t_sb in ((uq_sb, aq_sb, qT_sb), (uv_sb, av_sb, xT_sb)):
            for nh in range(2):
                psu = psA.tile([128, 512], F32, tag="psA", name="psu")
                for it in range(NT):
                    nc.tensor.matmul(
                        psu[0:RANK, :],
                        lhsT=a_sb[:, it, :],
                        rhs=t_sb[:, it, ts(nh, 512)],
                        start=(it == 0),
                        stop=(it == NT - 1),
                    )
                nc.scalar.copy(u_sb[:, ts(nh, 512)], psu[0:RANK, :])

        # qhT / khT: [c(out), n] = W^T-chunks @ tT, + LoRA up for q
        for oc in range(NT):
            for nh in range(2):
                psq = psA.tile([128, 512], F32, tag="psA", name="psq")
                for it in range(NT):
                    nc.tensor.matmul(
                        psq,
                        lhsT=wq_sb[:, it, ts(oc, 128)],
                        rhs=qT_sb[:, it, ts(nh, 512)],
                        start=(it == 0),
                        stop=False,
                    )
                nc.tensor.matmul(
                    psq,
                    lhsT=bq_sb[:, ts(oc, 128)],
                    rhs=uq_sb[:, ts(nh, 512)],
                    start=False,
                    stop=True,
                )
                nc.scalar.copy(qhT[:, oc, ts(nh, 512)], psq)

                psk = psA.tile([128, 512], F32, tag="psA", name="psk")
                for it in range(NT):
                    nc.tensor.matmul(
                        psk,
                        lhsT=wk_sb[:, it, ts(oc, 128)],
                        rhs=xT_sb[:, it, ts(nh, 512)],
                        start=(it == 0),
                        stop=(it == NT - 1),
                    )
                nc.scalar.copy(khT[:, oc, ts(nh, 512)], psk)

        # vh natural layout [n2, c] with LoRA, scattered into vha head blocks
        for tci in range(8):
            for oh in range(2):
                psv = psA.tile([128, 512], F32, tag="psA", name="psv")
                for it in range(NT):
                    nc.tensor.matmul(
                        psv[:, 0:384],
                        lhsT=xT_sb[:, it, ts(tci, 128)],
                        rhs=wv_sb[:, it, ds(oh * 384, 384)],
                        start=(it == 0),
                        stop=False,
                    )
                nc.tensor.matmul(
                    psv[:, 0:384],
                    lhsT=uv_sb[:, ts(tci, 128)],
                    rhs=bv_sb[:, ds(oh * 384, 384)],
                    start=False,
                    stop=True,
                )
                nc.scalar.copy(
                    vha[:, tci, oh * 6 : (oh + 1) * 6, 0:HD],
                    psv[:, 0:384].rearrange("p (h d) -> p h d", d=HD),
                )

    # ---------------- Phase B: attention per head ----------------
    # otn gets its own pool opened after ph1 released, so its SBUF doesn't
    # count against the phase-A high-water mark.
    otp = ctx.enter_context(tc.tile_pool(name="otp", bufs=1))
    otn = otp.tile([128, NT, N], F32R)  # normalized (attn @ V)^T, [c, n1]
    with tc.tile_pool(name="ptp", bufs=2) as ptp, \
         tc.tile_pool(name="misc", bufs=2) as misc, \
         tc.tile_pool(name="psST", bufs=2, space="PSUM") as psST, \
         tc.tile_pool(name="psAV", bufs=2, space="PSUM") as psAV, \
         tc.tile_pool(name="psRB", bufs=1, space="PSUM") as psRB:
        for h in range(HEADS):
            hp, hr = divmod(h, 2)
            hr *= HD
            pt = ptp.tile([128, 8, N], F32R, tag="pt", name="pt")
            # scores^T chunks + exp
            for c in range(8):
                pss = psST.tile([128, N], F32, tag="st", name="pss")
                for nh in range(2):
                    nc.tensor.matmul(
                        pss[:, ts(nh, 512)],
                        lhsT=khT[hr : hr + HD, hp, ts(c, 128)],
                        rhs=qhT[hr : hr + HD, hp, ts(nh, 512)],
                        start=True,
                        stop=True,
                    )
                nc.scalar.activation(pt[:, c, :], pss[:, :], AF.Exp)

            # attn@V with ones-augmented V: rows 0..63 = O^T, row 64 = sums
            r64 = misc.tile([HD + 1, N], F32, tag="r64", name="r64")
            nc.vector.memset(r64[0:HD, :], 0.0)
            av_ps = []
            for nh in range(2):
                pso = psAV.tile([128, 512], F32, tag="av", name="pso")
                for c in range(8):
                    nc.tensor.matmul(
                        pso[0 : HD + 1, :],
                        lhsT=vha[:, c, h, :],
                        rhs=pt[:, c, ts(nh, 512)],
                        start=(c == 0),
                        stop=(c == 7),
                    )
                nc.vector.reciprocal(r64[HD : HD + 1, ts(nh, 512)], pso[HD : HD + 1, :])
                av_ps.append(pso)

            # broadcast recip row across all 128 partitions (exact fp32 matmul),
            # then move to SBUF so DVE tensor_tensor has only one PSUM operand
            rb_ps = psRB.tile([128, N], F32, tag="rb", name="rb_ps")
            for nh in range(2):
                nc.tensor.matmul(
                    rb_ps[:, ts(nh, 512)],
                    lhsT=one65[:, :],
                    rhs=r64[:, ts(nh, 512)],
                    start=True,
                    stop=True,
                )
            rb = misc.tile([128, N], F32, tag="rb_sb", name="rb")
            nc.scalar.copy(rb[:, :], rb_ps[:, :])

            # normalize O^T into otn (odd heads need a partition shift via DMA)
            ott = None
            if h % 2 == 1:
                ott = misc.tile([HD, N], F32R, tag="ott", name="ott")
            for nh in range(2):
                dst = otn[0:HD, hp, ts(nh, 512)] if h % 2 == 0 else ott[:, ts(nh, 512)]
                nc.vector.tensor_tensor(
                    dst, av_ps[nh][0:HD, :], rb[0:HD, ts(nh, 512)], MULT
                )
            if h % 2 == 1:
                nc.sync.dma_start(otn[HD:128, hp, :], ott[:, :])

            # normalize attn^T and write out
            for c in range(8):
                nc.vector.tensor_tensor(pt[:, c, :], pt[:, c, :], rb[:, :], MULT)
                nc.sync.dma_start(attnT_d[h, ts(c, 128), :], pt[:, c, :])

        # ---------------- Phase C: output projection ----------------
        for oc in range(NT):
            for nh in range(2):
                psp = psAV.tile([128, 512], F32, tag="av", name="psp")
                for it in range(NT):
                    nc.tensor.matmul(
                        psp,
                        lhsT=wp_sb[:, it, ts(oc, 128)],
                        rhs=otn[:, it, ts(nh, 512)],
                        start=(it == 0),
                        stop=(it == NT - 1),
                    )
                ou = misc.tile([128, 512], F32, tag="ou", name="ou")
                nc.scalar.activation(
                    ou, psp, AF.Identity, bias=bp_sb[:, oc : oc + 1], scale=1.0
                )
                nc.sync.dma_start(outT_d[ts(oc, 128), ts(nh, 512)], ou)


_NC_CACHE = {}


def build_program():
    if "nc" in _NC_CACHE:
        return _NC_CACHE["nc"]
    nc = bacc.Bacc("TRN2", target_bir_lowering=False, debug=False)
    ins = {}
    for name, shape in [
        ("qT", [DIM, N]),
        ("xT", [DIM, N]),
        ("wqT", [DIM, DIM]),
        ("wkT", [DIM, DIM]),
        ("wvT", [DIM, DIM]),
        ("wpT", [DIM, DIM]),
        ("aqT", [DIM, RANK]),
        ("avT", [DIM, RANK]),
        ("bqT", [RANK, DIM]),
        ("bvT", [RANK, DIM]),
        ("bpP", [128, NT]),
    ]:
        dt_in = F32 if name == "bpP" else F32R
        ins[name] = nc.dram_tensor(name, shape, dt_in, kind="ExternalInput").ap()
    outs = {
        "attnT": nc.dram_tensor("attnT", [HEADS, N, N], F32R, kind="ExternalOutput").ap(),
        "outT": nc.dram_tensor("outT", [DIM, N], F32, kind="ExternalOutput").ap(),
    }
    with tile.TileContext(nc) as tc:
        emit_kernel(tc, ins, outs)
    nc.compile()
    _NC_CACHE["nc"] = nc
    return nc


def make_in_maps(q, x, Wq, Aq, Bq, Wk, Wv, Av, Bv, Wp, bp):
    f = lambda a: np.ascontiguousarray(np.asarray(a, dtype=np.float32))
    shared = {
        "wqT": f(Wq.T * SCALE),
        "wkT": f(Wk.T),
        "wvT": f(Wv.T),
        "wpT": f(Wp.T),
        "aqT": f(Aq.T),
        "avT": f(Av.T),
        "bqT": f(Bq.T * (SCALE * LORA_SCALE)),
        "bvT": f(Bv.T * LORA_SCALE),
        "bpP": f(np.asarray(bp).reshape(NT, 128).T),
    }
    in_maps = []
    for b in range(NCORES):
        m = dict(shared)
        m["qT"] = f(np.asarray(q[b]).T)
        m["xT"] = f(np.asarray(x[b]).T)
        in_maps.append(m)
    return in_maps


def kernel(q, x, Wq, Aq, Bq, Wk, Wv, Av, Bv, Wp, bp):
    nc = build_program()
    in_maps = make_in_maps(q, x, Wq, Aq, Bq, Wk, Wv, Av, Bv, Wp, bp)
    res = run_bass_kernel_spmd(nc, in_maps, list(range(NCORES)))
    attn = np.stack([res.results[b]["attnT"] for b in range(NCORES)], axis=0)
    attn = attn.transpose(0, 1, 3, 2)  # [B, H, n2, n1] -> [B, H, n1, n2]
    out = np.stack([res.results[b]["outT"].T for b in range(NCORES)], axis=0)
    return out, attn
